# revision 18
# baseline (speedup 1.0000x reference)
"""AttentionWithSelfAblation TRN2 kernel.

Reference computation (B=4, S=2048, H=1024, nh=16, hd=64, window=256):
    q = x @ Wq.T ; k = x_clean @ Wk.T ; v = x_clean @ Wv.T   (per-head split)
    scores = q @ k.T  (NO 1/sqrt(hd) scaling)
    local causal mask: key j visible to query i iff i-255 <= j <= i
    attn = softmax(scores) ; ctx = attn @ v  (merge heads)
    out = (ctx * ablation_mask) @ Wo.T + bo

Sharding: pure data/sequence parallel over 8 cores: core c = (batch c//2,
sequence half c%2 of 1024 queries). Keys/values need a 256-halo to the left;
the first half uses zero-padding + masks instead. No collectives.

Per-core device pipeline (weights stream as halves, 3-deep rotation so the
next phase's weights prefetch during the current phase's compute):
  phase Q : xT chunks streamed -> qT[o,s] f32r
  phase KV: xcT chunks streamed (twice, once per weight half) ->
            kT[o,s] f32r + v[s,o] bf16 (o augmented with ones per head)
  phase A : per (qpair of 256 queries, head): scoresT[sk,sq] =
            maskneg (PE-injected bf16) + kT_h.T @ qT_h (f32r) ; exp on ACT
            -> bf16 ; ctx-MM v.T @ exp (aug ones row = denominator) ;
            denominator copy on ACT, reciprocal + K=1 broadcast-MMs ;
            normalize * ablation (DVE) ; out-proj (PE) + bias (ACT) -> bf16.

The q/k score path stays f32r end-to-end (bf16 there fails the 2e-2 gate;
mixed bf16-stationary x f32r-moving matmul is rejected by walrus).
Host does all layout transposes/casts (free) and unshards by concatenation.
"""

import numpy as np
import ml_dtypes

from concourse import bacc
import concourse.tile as tile
import concourse.mybir as mybir
from concourse.bass_utils import run_bass_kernel_spmd

B, S, H = 4, 2048, 1024
NH, HD = 16, 64
W = 256  # window
SL = 1024  # per-core sequence chunk
SKL = SL + W  # keys incl halo
NQP = SL // 256  # qpairs of 256 queries
NKT = 4  # k-tiles of 128 per qpair
NC = 8  # cores

F32 = mybir.dt.float32
F32R = mybir.dt.float32r
BF16 = mybir.dt.bfloat16
EXP = mybir.ActivationFunctionType.Exp
IDENT = mybir.ActivationFunctionType.Identity

_compiled = None


def _build():
    nc = bacc.Bacc("TRN2", target_bir_lowering=False, debug=False)

    xT = nc.dram_tensor("xT", [H, SL], F32R, kind="ExternalInput")
    xcT = nc.dram_tensor("xcT", [H, SKL], F32R, kind="ExternalInput")
    ablT = nc.dram_tensor("ablT", [H, SL], F32, kind="ExternalInput")
    WqT = nc.dram_tensor("WqT", [H, H], F32R, kind="ExternalInput")
    WkT = nc.dram_tensor("WkT", [H, H], F32R, kind="ExternalInput")
    WvT = nc.dram_tensor("WvT", [H, H], F32R, kind="ExternalInput")
    WoT = nc.dram_tensor("WoT", [H, H], F32R, kind="ExternalInput")
    bo = nc.dram_tensor("bo", [H], F32, kind="ExternalInput")
    # masks[set, kt, sk, sq]: set 0 = standard (qp>=1), set 1 = qp==0 variant
    masks = nc.dram_tensor("masks", [2, NKT, 128, 256], BF16, kind="ExternalInput")
    ident_in = nc.dram_tensor("ident_in", [128, 128], BF16, kind="ExternalInput")
    pmask_in = nc.dram_tensor("pmask_in", [1, 2, 128], F32R, kind="ExternalInput")
    outT = nc.dram_tensor("outT", [H, SL], BF16, kind="ExternalOutput")

    xT_d = xT.rearrange("(c p) s -> p c s", p=128)
    xcT_d = xcT.rearrange("(c p) s -> p c s", p=128)
    ablT_d = ablT.rearrange("(t p) s -> p t s", p=128)
    outT_d = outT.rearrange("(t p) s -> p t s", p=128)

    with tile.TileContext(nc) as tc:
        with (
            tc.tile_pool(name="consts", bufs=1) as consts,
            tc.tile_pool(name="big", bufs=1) as big,
            tc.tile_pool(name="wpool", bufs=3) as wpool,
            tc.tile_pool(name="outp", bufs=3) as outpool,
            tc.tile_pool(name="ps512", bufs=2, space="PSUM") as ps512,
        ):

            qT_sb = big.tile([128, 8, SL], F32R)
            kT_sb = big.tile([128, 8, SKL], F32R)
            v_sb = big.tile([128, 10, 16 * 65], BF16)
            xc0_sb = big.tile([128, 8, 256], F32R, name="xc0")

            def load_weight_half(dram, hf):
                """o-columns [hf*512, (hf+1)*512) of a transposed weight."""
                w_sb = wpool.tile(
                    [128, 8, 512], F32R, name=f"w_{dram.name}_{hf}", tag="w"
                )
                for c in range(8):
                    nc.sync.dma_start(
                        w_sb[:, c, :],
                        dram.rearrange("(c p) o -> p c o", p=128)[
                            :, c, hf * 512 : (hf + 1) * 512
                        ],
                    )
                return w_sb

            # ones columns of the augmented v (slot 64 of each head's 65):
            # memset a contiguous scratch then strided-copy into place
            v_aug = v_sb[:].rearrange("p t (h e) -> p t h e", e=65)
            ones_scratch = consts.tile([128, 160], BF16)
            nc.vector.memset(ones_scratch[:], 1.0)
            nc.vector.tensor_copy(
                v_aug[:, :, :, 64],
                ones_scratch[:].rearrange("p (t h) -> p t h", t=10),
            )

            # ---- phases Q + KV share one streaming pool: the first xc
            # chunk's DMA starts as soon as an x buffer frees mid-phase-Q ----
            with tc.tile_pool(name="xs", bufs=2) as xspool:
                for hf in range(2):
                    wq_sb = load_weight_half(WqT, hf)
                    for ci in range(SL // 512):
                        x_s = xspool.tile(
                            [128, 8, 512], F32R, name=f"x_{hf}_{ci}", tag="xs"
                        )
                        for c in range(8):
                            nc.sync.dma_start(
                                x_s[:, c, :], xT_d[:, c, ci * 512 : (ci + 1) * 512]
                            )
                        if hf == 0 and ci == 0:
                            # c-major over 4 banks: compute starts after the
                            # first c-chunk lands instead of the whole wave
                            with tc.tile_pool(
                                name="psq", bufs=1, space="PSUM"
                            ) as psqp:
                                psQ = psqp.tile([128, 4, 512], F32, name="psq0")
                                for c in range(8):
                                    for oi in range(4):
                                        nc.tensor.matmul(
                                            psQ[:, oi, :],
                                            wq_sb[:, c, oi * 128 : (oi + 1) * 128],
                                            x_s[:, c, :],
                                            start=(c == 0),
                                            stop=(c == 7),
                                            skip_group_check=True,
                                        )
                                for oi in range(4):
                                    nc.vector.tensor_copy(
                                        qT_sb[:, oi, 0:512], psQ[:, oi, :]
                                    )
                        else:
                          for oi in range(4):
                            ot = hf * 4 + oi
                            ps = ps512.tile([128, 512], F32, tag="ps512")
                            for c in range(8):
                                nc.tensor.matmul(
                                    ps[:],
                                    wq_sb[:, c, oi * 128 : (oi + 1) * 128],
                                    x_s[:, c, :],
                                    start=(c == 0),
                                    stop=(c == 7),
                                )
                            nc.vector.tensor_copy(
                                qT_sb[:, ot, ci * 512 : (ci + 1) * 512], ps[:]
                            )

                # first KV chunk rides along during phase-Q compute, when
                # the DMA queues have spare bandwidth
                for c in range(8):
                    nc.sync.dma_start(xc0_sb[:, c, :], xcT_d[:, c, 1024:1280])

                # ---- phase KV: kT[o,s] f32r + v[s,o] bf16 (o aug per head) ----
                kv_chunks = [(1024, 256), (0, 512), (512, 512)]
                for hf in range(2):
                    wk_sb = load_weight_half(WkT, hf)
                    wv_sb = load_weight_half(WvT, hf)
                    for ci, (s0c, snc) in enumerate(kv_chunks):
                        if ci == 0:
                            xc_s = xc0_sb  # preloaded during phase Q
                        else:
                            xc_s = xspool.tile(
                                [128, 8, 512], F32R, name=f"xc_{hf}_{ci}", tag="xs"
                            )
                            for c in range(8):
                                nc.sync.dma_start(
                                    xc_s[:, c, :snc], xcT_d[:, c, s0c : s0c + snc]
                                )
                        for oi in range(4):
                            ot = hf * 4 + oi
                            ps = ps512.tile([128, 512], F32, tag="ps512")
                            for c in range(8):
                                nc.tensor.matmul(
                                    ps[:, :snc],
                                    wk_sb[:, c, oi * 128 : (oi + 1) * 128],
                                    xc_s[:, c, :snc],
                                    start=(c == 0),
                                    stop=(c == 7),
                                )
                            nc.scalar.copy(
                                kT_sb[:, ot, s0c : s0c + snc], ps[:, :snc]
                            )
                        for sti in range(snc // 128):
                            st = s0c // 128 + sti
                            ps = ps512.tile([128, 512], F32, tag="ps512")
                            for c in range(8):
                                nc.tensor.matmul(
                                    ps[:],
                                    xc_s[:, c, sti * 128 : (sti + 1) * 128],
                                    wv_sb[:, c, :],
                                    start=(c == 0),
                                    stop=(c == 7),
                                )
                            nc.scalar.copy(
                                v_aug[:, st, hf * 8 : (hf + 1) * 8, 0:64],
                                ps[:].rearrange("p (h e) -> p h e", e=64),
                            )

            # ---- attention constants (loaded late: not needed before phase A,
            # and early emission delays the first projection DMAs) ----
            ident = consts.tile([128, 128], BF16)
            nc.sync.dma_start(ident[:], ident_in[:])
            pmask = consts.tile([1, 2, 128], F32R)
            nc.sync.dma_start(pmask[:], pmask_in[:])
            bo_sb = consts.tile([128, 8], F32)
            nc.sync.dma_start(bo_sb[:], bo.rearrange("(t p) -> p t", p=128))
            mask_sb = consts.tile([128, 2, NKT, 256], BF16)
            for ms in range(2):
                nc.sync.dma_start(
                    mask_sb[:, ms, :, :],
                    masks.rearrange("s t k q -> k s t q")[:, ms, :, :],
                )

            # ---- phase A: attention + out-projection per qpair ----
            wo_hs = [load_weight_half(WoT, hf) for hf in range(2)]
            with (
                tc.tile_pool(name="exp", bufs=3) as exppool,
                tc.tile_pool(name="recip", bufs=3) as recippool,
                tc.tile_pool(name="abl", bufs=3) as ablpool,
                tc.tile_pool(name="ctxs", bufs=1) as ctxpool,
                tc.tile_pool(name="ps_sc", bufs=2, space="PSUM") as ps_sc,
                tc.tile_pool(name="ps_ctx", bufs=2, space="PSUM") as ps_ctx,
            ):
                for qg in range(NQP // 2):
                  ctx_sb = ctxpool.tile(
                      [128, 8, 512], F32R, name=f"ctx_{qg}", tag="ctx"
                  )
                  for qph in range(2):
                    qp = qg * 2 + qph
                    qsl = slice(qph * 256, qph * 256 + 256)
                    ms = 1 if qp == 0 else 0
                    for t in range(NH // 2):  # head pair
                        pss = [
                            ps_sc.tile(
                                [128, NKT, 256], F32,
                                name=f"sc_{qp}_{2 * t + par}", tag="sc",
                            )
                            for par in range(2)
                        ]
                        exps = []
                        # injects first, then the qk matmuls kt-major so the
                        # even (rows 0:64) / odd (rows 64:128) head pairs are
                        # adjacent -- disjoint row-groups execute concurrently
                        for par in range(2):
                            ps = pss[par]
                            for kg in range(2):  # one mask inject per 2 k-tiles
                                nc.tensor.matmul(
                                    ps[:, kg * 2 : kg * 2 + 2, :],
                                    ident[:],
                                    mask_sb[:, ms, kg * 2 : kg * 2 + 2, :],
                                    start=True,
                                    stop=False,
                                    skip_group_check=True,
                                )
                        for kt in range(NKT):
                            lj0 = qp * 256 + kt * 128
                            for par in range(2):
                                hsl = slice(par * 64, par * 64 + 64)
                                nc.tensor.matmul(
                                    pss[par][:, kt, :],
                                    kT_sb[hsl, t, lj0 : lj0 + 128],
                                    qT_sb[hsl, t, qp * 256 : qp * 256 + 256],
                                    start=False,
                                    stop=True,
                                    skip_group_check=True,
                                )
                        for par in range(2):
                            exp_sb = exppool.tile(
                                [128, NKT, 256], BF16,
                                name=f"exp_{qp}_{2 * t + par}", tag="exp",
                            )
                            exps.append(exp_sb)
                            nc.scalar.activation(exp_sb[:], pss[par][:], EXP)
                        psc = ps_ctx.tile(
                            [65, 2, 256], F32, name=f"ctxp_{qp}_{t}", tag="ctxp"
                        )
                        for par in range(2):
                            h = 2 * t + par
                            for kt in range(NKT):
                                nc.tensor.matmul(
                                    psc[:, par, :],
                                    v_sb[:, qp * 2 + kt, h * 65 : h * 65 + 65],
                                    exps[par][:, kt, :],
                                    start=(kt == 0),
                                    stop=(kt == NKT - 1),
                                )
                        recf = recippool.tile(
                            [1, 2, 512], F32, name=f"recf_{qp}_{t}", tag="recf"
                        )
                        rec = recippool.tile(
                            [1, 2, 256], F32R, name=f"rec_{qp}_{t}", tag="rec"
                        )
                        # denominator copy off PSUM on ACT (frees the DVE)
                        nc.scalar.copy(
                            recf[:, 0, :].rearrange("p (a q) -> p a q", a=2),
                            psc[64:65, :, :],
                        )
                        nc.vector.reciprocal_approx_fast(recf[:, 1, :], recf[:, 0, :])
                        nc.vector.tensor_copy(
                            rec[:], recf[:, 1, :].rearrange("p (a q) -> p a q", a=2)
                        )
                        # drain pair: even head -> parts 0:64, odd -> 64:128
                        nc.vector.tensor_copy(ctx_sb[0:64, t, qsl], psc[0:64, 0, :])
                        nc.vector.tensor_copy(ctx_sb[64:128, t, qsl], psc[0:64, 1, :])
                        # normalize + ablate inline (spreads the PE bcast MMs
                        # across the t loop instead of bunching at qp end)
                        psb = ps512.tile(
                            [128, 512], F32, name=f"bc_{qp}_{t}", tag="ps512"
                        )
                        for par in range(2):
                            nc.tensor.matmul(
                                psb[:, :256],
                                pmask[:, par, :],
                                rec[:, par, :],
                                start=(par == 0),
                                stop=(par == 1),
                            )
                        abl_sb = ablpool.tile(
                            [128, 256], F32, name=f"abl_{qp}_{t}", tag="abl"
                        )
                        nc.sync.dma_start(
                            abl_sb[:], ablT_d[:, t, qp * 256 : qp * 256 + 256]
                        )
                        nc.vector.tensor_mul(
                            ctx_sb[:, t, qsl], ctx_sb[:, t, qsl], psb[:, :256]
                        )
                        nc.vector.tensor_mul(
                            ctx_sb[:, t, qsl], ctx_sb[:, t, qsl], abl_sb[:]
                        )

                  # out projection for this pair of qpairs (N=512)
                  for ot in range(8):
                      wo_sb = wo_hs[ot // 4]
                      oi = ot % 4
                      ps = ps512.tile(
                          [128, 512], F32, name=f"op_{qg}_{ot}", tag="ps512"
                      )
                      for c in range(8):
                          nc.tensor.matmul(
                              ps[:],
                              wo_sb[:, c, oi * 128 : (oi + 1) * 128],
                              ctx_sb[:, c, :],
                              start=(c == 0),
                              stop=(c == 7),
                          )
                      o_sb = outpool.tile(
                          [128, 512], BF16, name=f"out_{qg}_{ot}", tag="outp"
                      )
                      nc.scalar.activation(
                          o_sb[:], ps[:], IDENT, bias=bo_sb[:, ot : ot + 1]
                      )
                      nc.sync.dma_start(
                          outT_d[:, ot, qg * 512 : qg * 512 + 512], o_sb[:]
                      )
    nc.compile()
    return nc


def kernel(x, x_clean, ablation_mask, Wq, Wk, Wv, Wo, bo):
    global _compiled
    x = np.asarray(x, np.float32)
    x_clean = np.asarray(x_clean, np.float32)
    ablation_mask = np.asarray(ablation_mask, np.float32)
    WqT = np.ascontiguousarray(np.asarray(Wq, np.float32).T)
    WkT = np.ascontiguousarray(np.asarray(Wk, np.float32).T)
    WvT = np.ascontiguousarray(np.asarray(Wv, np.float32).T)
    WoT = np.ascontiguousarray(np.asarray(Wo, np.float32).T)
    bo = np.asarray(bo, np.float32)

    ident = np.eye(128, dtype=ml_dtypes.bfloat16)
    pmask = np.zeros((1, 2, 128), np.float32)
    pmask[0, 0, 0:64] = 1.0
    pmask[0, 1, 64:128] = 1.0

    # masks: include iff 1 <= kt*128 + r - a <= 256 ; set 1 adds qp==0 edge
    r = np.arange(128)[:, None]
    a = np.arange(256)[None, :]
    masks_by_half = []
    for half in range(2):
        m = np.empty((2, NKT, 128, 256), np.float32)
        for kt in range(NKT):
            d = kt * 128 + r - a
            inc = (d >= 1) & (d <= 256)
            m[0, kt] = np.where(inc, 0.0, -1e30)
            inc_edge = inc & ((kt * 128 + r) >= 256) if half == 0 else inc
            m[1, kt] = np.where(inc_edge, 0.0, -1e30)
        masks_by_half.append(m.astype(ml_dtypes.bfloat16))

    in_maps = []
    for c in range(NC):
        b, half = c // 2, c % 2
        s0 = half * SL
        xTc = np.ascontiguousarray(x[b, s0 : s0 + SL].T)
        xc = np.zeros((SKL, H), np.float32)
        lo = max(0, s0 - W)
        xc[W - (s0 - lo) :] = x_clean[b, lo : s0 + SL]
        xcTc = np.ascontiguousarray(xc.T)
        ablTc = np.ascontiguousarray(ablation_mask[b, s0 : s0 + SL].T)
        in_maps.append(
            {
                "xT": xTc,
                "xcT": xcTc,
                "ablT": ablTc,
                "WqT": WqT,
                "WkT": WkT,
                "WvT": WvT,
                "WoT": WoT,
                "bo": bo,
                "masks": masks_by_half[half],
                "ident_in": ident,
                "pmask_in": pmask,
            }
        )

    if _compiled is None:
        _compiled = _build()
    res = run_bass_kernel_spmd(
        _compiled, in_maps, core_ids=list(range(NC)), trace=False
    )

    out = np.empty((B, S, H), np.float32)
    for c in range(NC):
        b, half = c // 2, c % 2
        out[b, half * SL : (half + 1) * SL] = (
            res.results[c]["outT"].T.astype(np.float32)
        )
    return out


# revision 19
# speedup vs baseline: 1.1943x; 1.1943x over previous
"""AttentionWithSelfAblation TRN2 kernel.

Reference computation (B=4, S=2048, H=1024, nh=16, hd=64, window=256):
    q = x @ Wq.T ; k = x_clean @ Wk.T ; v = x_clean @ Wv.T   (per-head split)
    scores = q @ k.T  (NO 1/sqrt(hd) scaling)
    local causal mask: key j visible to query i iff i-255 <= j <= i
    attn = softmax(scores) ; ctx = attn @ v  (merge heads)
    out = (ctx * ablation_mask) @ Wo.T + bo

Sharding: pure data/sequence parallel over 8 cores: core c = (batch c//2,
sequence half c%2 of 1024 queries). Keys/values need a 256-halo to the left;
the first half uses zero-padding + masks instead. No collectives.

Per-core device pipeline (weights stream as halves, 3-deep rotation so the
next phase's weights prefetch during the current phase's compute):
  phase Q : xT chunks streamed -> qT[o,s] f32r
  phase KV: xcT chunks streamed (twice, once per weight half) ->
            kT[o,s] f32r + v[s,o] bf16 (o augmented with ones per head)
  phase A : per (qpair of 256 queries, head): scoresT[sk,sq] =
            maskneg (PE-injected bf16) + kT_h.T @ qT_h (f32r) ; exp on ACT
            -> bf16 ; ctx-MM v.T @ exp (aug ones row = denominator) ;
            denominator copy on ACT, reciprocal + K=1 broadcast-MMs ;
            normalize * ablation (DVE) ; out-proj (PE) + bias (ACT) -> bf16.

The q/k score path stays f32r end-to-end (bf16 there fails the 2e-2 gate;
mixed bf16-stationary x f32r-moving matmul is rejected by walrus).
Host does all layout transposes/casts (free) and unshards by concatenation.
"""

import numpy as np
import ml_dtypes

from concourse import bacc
import concourse.tile as tile
import concourse.mybir as mybir
from concourse.bass_utils import run_bass_kernel_spmd

B, S, H = 4, 2048, 1024
NH, HD = 16, 64
W = 256  # window
SL = 1024  # per-core sequence chunk
SKL = SL + W  # keys incl halo
NQP = SL // 256  # qpairs of 256 queries
NKT = 4  # k-tiles of 128 per qpair
NC = 8  # cores

F32 = mybir.dt.float32
F32R = mybir.dt.float32r
BF16 = mybir.dt.bfloat16
EXP = mybir.ActivationFunctionType.Exp
IDENT = mybir.ActivationFunctionType.Identity

_compiled = None


def _build():
    nc = bacc.Bacc("TRN2", target_bir_lowering=False, debug=False)

    xT = nc.dram_tensor("xT", [H, SL], F32R, kind="ExternalInput")
    xcT = nc.dram_tensor("xcT", [H, SKL], F32R, kind="ExternalInput")
    ablT = nc.dram_tensor("ablT", [H, SL], F32, kind="ExternalInput")
    WqT = nc.dram_tensor("WqT", [H, H], F32R, kind="ExternalInput")
    WkT = nc.dram_tensor("WkT", [H, H], F32R, kind="ExternalInput")
    WvT = nc.dram_tensor("WvT", [H, H], F32R, kind="ExternalInput")
    WoT = nc.dram_tensor("WoT", [H, H], F32R, kind="ExternalInput")
    bo = nc.dram_tensor("bo", [H], F32, kind="ExternalInput")
    # masks[set, kt, sk, sq]: set 0 = standard (qp>=1), set 1 = qp==0 variant
    masks = nc.dram_tensor("masks", [2, NKT, 128, 256], BF16, kind="ExternalInput")
    ident_in = nc.dram_tensor("ident_in", [128, 128], BF16, kind="ExternalInput")
    pmask_in = nc.dram_tensor("pmask_in", [1, 2, 128], F32R, kind="ExternalInput")
    outT = nc.dram_tensor("outT", [H, SL], BF16, kind="ExternalOutput")

    xT_d = xT.rearrange("(c p) s -> p c s", p=128)
    xcT_d = xcT.rearrange("(c p) s -> p c s", p=128)
    ablT_d = ablT.rearrange("(t p) s -> p t s", p=128)
    outT_d = outT.rearrange("(t p) s -> p t s", p=128)

    with tile.TileContext(nc) as tc:
        with (
            tc.tile_pool(name="consts", bufs=1) as consts,
            tc.tile_pool(name="big", bufs=1) as big,
            tc.tile_pool(name="wpool", bufs=3) as wpool,
            tc.tile_pool(name="outp", bufs=3) as outpool,
            tc.tile_pool(name="ps512", bufs=2, space="PSUM") as ps512,
        ):

            qT_sb = big.tile([128, 8, SL], F32R)
            kT_sb = big.tile([128, 8, SKL], F32R)
            v_sb = big.tile([128, 10, 16 * 65], BF16)
            xc0_sb = big.tile([128, 8, 256], F32R, name="xc0")

            def load_weight_half(dram, hf):
                """o-columns [hf*512, (hf+1)*512) of a transposed weight."""
                w_sb = wpool.tile(
                    [128, 8, 512], F32R, name=f"w_{dram.name}_{hf}", tag="w"
                )
                for c in range(8):
                    nc.sync.dma_start(
                        w_sb[:, c, :],
                        dram.rearrange("(c p) o -> p c o", p=128)[
                            :, c, hf * 512 : (hf + 1) * 512
                        ],
                    )
                return w_sb

            # ones columns of the augmented v (slot 64 of each head's 65):
            # memset a contiguous scratch then strided-copy into place
            v_aug = v_sb[:].rearrange("p t (h e) -> p t h e", e=65)
            ones_scratch = consts.tile([128, 160], BF16)
            nc.vector.memset(ones_scratch[:], 1.0)
            nc.vector.tensor_copy(
                v_aug[:, :, :, 64],
                ones_scratch[:].rearrange("p (t h) -> p t h", t=10),
            )

            # ---- phases Q + KV share one streaming pool: the first xc
            # chunk's DMA starts as soon as an x buffer frees mid-phase-Q ----
            with tc.tile_pool(name="xs", bufs=2) as xspool:
                for hf in range(2):
                    wq_sb = load_weight_half(WqT, hf)
                    for ci in range(SL // 512):
                        x_s = xspool.tile(
                            [128, 8, 512], F32R, name=f"x_{hf}_{ci}", tag="xs"
                        )
                        for c in range(8):
                            nc.sync.dma_start(
                                x_s[:, c, :], xT_d[:, c, ci * 512 : (ci + 1) * 512]
                            )
                        if hf == 0 and ci == 0:
                            # c-major over 4 banks: compute starts after the
                            # first c-chunk lands instead of the whole wave
                            with tc.tile_pool(
                                name="psq", bufs=1, space="PSUM"
                            ) as psqp:
                                psQ = psqp.tile([128, 4, 512], F32, name="psq0")
                                for c in range(8):
                                    for oi in range(4):
                                        nc.tensor.matmul(
                                            psQ[:, oi, :],
                                            wq_sb[:, c, oi * 128 : (oi + 1) * 128],
                                            x_s[:, c, :],
                                            start=(c == 0),
                                            stop=(c == 7),
                                            skip_group_check=True,
                                        )
                                for oi in range(4):
                                    nc.vector.tensor_copy(
                                        qT_sb[:, oi, 0:512], psQ[:, oi, :]
                                    )
                        else:
                          for oi in range(4):
                            ot = hf * 4 + oi
                            ps = ps512.tile([128, 512], F32, tag="ps512")
                            for c in range(8):
                                nc.tensor.matmul(
                                    ps[:],
                                    wq_sb[:, c, oi * 128 : (oi + 1) * 128],
                                    x_s[:, c, :],
                                    start=(c == 0),
                                    stop=(c == 7),
                                )
                            nc.vector.tensor_copy(
                                qT_sb[:, ot, ci * 512 : (ci + 1) * 512], ps[:]
                            )

                # first KV chunk rides along during phase-Q compute, when
                # the DMA queues have spare bandwidth
                for c in range(8):
                    nc.sync.dma_start(xc0_sb[:, c, :], xcT_d[:, c, 1024:1280])

                # ---- phase KV: kT[o,s] f32r + v[s,o] bf16 (o aug per head) ----
                kv_chunks = [(1024, 256), (0, 512), (512, 512)]
                for hf in range(2):
                    wk_sb = load_weight_half(WkT, hf)
                    wv_sb = load_weight_half(WvT, hf)
                    for ci, (s0c, snc) in enumerate(kv_chunks):
                        if ci == 0:
                            xc_s = xc0_sb  # preloaded during phase Q
                        else:
                            xc_s = xspool.tile(
                                [128, 8, 512], F32R, name=f"xc_{hf}_{ci}", tag="xs"
                            )
                            for c in range(8):
                                nc.sync.dma_start(
                                    xc_s[:, c, :snc], xcT_d[:, c, s0c : s0c + snc]
                                )
                        for oi in range(4):
                            ot = hf * 4 + oi
                            ps = ps512.tile([128, 512], F32, tag="ps512")
                            for c in range(8):
                                nc.tensor.matmul(
                                    ps[:, :snc],
                                    wk_sb[:, c, oi * 128 : (oi + 1) * 128],
                                    xc_s[:, c, :snc],
                                    start=(c == 0),
                                    stop=(c == 7),
                                )
                            nc.scalar.copy(
                                kT_sb[:, ot, s0c : s0c + snc], ps[:, :snc]
                            )
                        for sti in range(snc // 128):
                            st = s0c // 128 + sti
                            ps = ps512.tile([128, 512], F32, tag="ps512")
                            for c in range(8):
                                nc.tensor.matmul(
                                    ps[:],
                                    xc_s[:, c, sti * 128 : (sti + 1) * 128],
                                    wv_sb[:, c, :],
                                    start=(c == 0),
                                    stop=(c == 7),
                                )
                            nc.scalar.copy(
                                v_aug[:, st, hf * 8 : (hf + 1) * 8, 0:64],
                                ps[:].rearrange("p (h e) -> p h e", e=64),
                            )

            # ---- attention constants (loaded late: not needed before phase A,
            # and early emission delays the first projection DMAs) ----
            ident = consts.tile([128, 128], BF16)
            nc.sync.dma_start(ident[:], ident_in[:])
            pmask = consts.tile([1, 2, 128], F32R)
            nc.sync.dma_start(pmask[:], pmask_in[:])
            bo_sb = consts.tile([128, 8], F32)
            nc.sync.dma_start(bo_sb[:], bo.rearrange("(t p) -> p t", p=128))
            mask_sb = consts.tile([128, 2, NKT, 256], BF16)
            for ms in range(2):
                nc.sync.dma_start(
                    mask_sb[:, ms, :, :],
                    masks.rearrange("s t k q -> k s t q")[:, ms, :, :],
                )

            # ---- phase A: attention + out-projection per qpair ----
            wo_hs = [load_weight_half(WoT, hf) for hf in range(2)]
            with (
                tc.tile_pool(name="exp", bufs=3) as exppool,
                tc.tile_pool(name="recip", bufs=3) as recippool,
                tc.tile_pool(name="abl", bufs=3) as ablpool,
                tc.tile_pool(name="ctxs", bufs=1) as ctxpool,
                tc.tile_pool(name="ps_sc", bufs=2, space="PSUM") as ps_sc,
                tc.tile_pool(name="ps_ctx", bufs=2, space="PSUM") as ps_ctx,
            ):
                for qg in range(NQP // 2):
                  ctx_sb = ctxpool.tile(
                      [128, 8, 512], F32R, name=f"ctx_{qg}", tag="ctx"
                  )
                  for qph in range(2):
                    qp = qg * 2 + qph
                    qsl = slice(qph * 256, qph * 256 + 256)
                    ms = 1 if qp == 0 else 0
                    for t in range(NH // 2):  # head pair
                        pss = [
                            ps_sc.tile(
                                [128, NKT, 256], F32,
                                name=f"sc_{qp}_{2 * t + par}", tag="sc",
                            )
                            for par in range(2)
                        ]
                        exps = []
                        # per par: inject mask, qk matmuls, then exp --
                        # exp of par 0 overlaps the par-1 matmuls, and the
                        # even/odd 64-row groups overlap on the PE
                        for par in range(2):
                            ps = pss[par]
                            for kg in range(2):  # one mask inject per 2 k-tiles
                                nc.tensor.matmul(
                                    ps[:, kg * 2 : kg * 2 + 2, :],
                                    ident[:],
                                    mask_sb[:, ms, kg * 2 : kg * 2 + 2, :],
                                    start=True,
                                    stop=False,
                                    skip_group_check=True,
                                )
                            hsl = slice(par * 64, par * 64 + 64)
                            for kt in range(NKT):
                                lj0 = qp * 256 + kt * 128
                                nc.tensor.matmul(
                                    ps[:, kt, :],
                                    kT_sb[hsl, t, lj0 : lj0 + 128],
                                    qT_sb[hsl, t, qp * 256 : qp * 256 + 256],
                                    start=False,
                                    stop=True,
                                    skip_group_check=True,
                                )
                            exp_sb = exppool.tile(
                                [128, NKT, 256], BF16,
                                name=f"exp_{qp}_{2 * t + par}", tag="exp",
                            )
                            exps.append(exp_sb)
                            nc.scalar.activation(exp_sb[:], ps[:], EXP)
                        psc = ps_ctx.tile(
                            [65, 2, 256], F32, name=f"ctxp_{qp}_{t}", tag="ctxp"
                        )
                        for par in range(2):
                            h = 2 * t + par
                            for kt in range(NKT):
                                nc.tensor.matmul(
                                    psc[:, par, :],
                                    v_sb[:, qp * 2 + kt, h * 65 : h * 65 + 65],
                                    exps[par][:, kt, :],
                                    start=(kt == 0),
                                    stop=(kt == NKT - 1),
                                )
                        recf = recippool.tile(
                            [1, 2, 512], F32, name=f"recf_{qp}_{t}", tag="recf"
                        )
                        rec = recippool.tile(
                            [1, 2, 256], F32R, name=f"rec_{qp}_{t}", tag="rec"
                        )
                        # denominator copy off PSUM on ACT (frees the DVE)
                        nc.scalar.copy(
                            recf[:, 0, :].rearrange("p (a q) -> p a q", a=2),
                            psc[64:65, :, :],
                        )
                        nc.vector.reciprocal_approx_fast(recf[:, 1, :], recf[:, 0, :])
                        nc.vector.tensor_copy(
                            rec[:], recf[:, 1, :].rearrange("p (a q) -> p a q", a=2)
                        )
                        # drain pair: even head -> parts 0:64, odd -> 64:128
                        nc.vector.tensor_copy(ctx_sb[0:64, t, qsl], psc[0:64, 0, :])
                        nc.vector.tensor_copy(ctx_sb[64:128, t, qsl], psc[0:64, 1, :])
                        # normalize + ablate inline (spreads the PE bcast MMs
                        # across the t loop instead of bunching at qp end)
                        psb = ps512.tile(
                            [128, 512], F32, name=f"bc_{qp}_{t}", tag="ps512"
                        )
                        for par in range(2):
                            nc.tensor.matmul(
                                psb[:, :256],
                                pmask[:, par, :],
                                rec[:, par, :],
                                start=(par == 0),
                                stop=(par == 1),
                            )
                        abl_sb = ablpool.tile(
                            [128, 256], F32, name=f"abl_{qp}_{t}", tag="abl"
                        )
                        nc.sync.dma_start(
                            abl_sb[:], ablT_d[:, t, qp * 256 : qp * 256 + 256]
                        )
                        nc.vector.tensor_mul(
                            ctx_sb[:, t, qsl], ctx_sb[:, t, qsl], psb[:, :256]
                        )
                        nc.vector.tensor_mul(
                            ctx_sb[:, t, qsl], ctx_sb[:, t, qsl], abl_sb[:]
                        )

                  # out projection for this pair of qpairs (N=512)
                  for ot in range(8):
                      wo_sb = wo_hs[ot // 4]
                      oi = ot % 4
                      ps = ps512.tile(
                          [128, 512], F32, name=f"op_{qg}_{ot}", tag="ps512"
                      )
                      for c in range(8):
                          nc.tensor.matmul(
                              ps[:],
                              wo_sb[:, c, oi * 128 : (oi + 1) * 128],
                              ctx_sb[:, c, :],
                              start=(c == 0),
                              stop=(c == 7),
                          )
                      o_sb = outpool.tile(
                          [128, 512], BF16, name=f"out_{qg}_{ot}", tag="outp"
                      )
                      nc.scalar.activation(
                          o_sb[:], ps[:], IDENT, bias=bo_sb[:, ot : ot + 1]
                      )
                      nc.sync.dma_start(
                          outT_d[:, ot, qg * 512 : qg * 512 + 512], o_sb[:]
                      )
    nc.compile()
    return nc


def kernel(x, x_clean, ablation_mask, Wq, Wk, Wv, Wo, bo):
    global _compiled
    x = np.asarray(x, np.float32)
    x_clean = np.asarray(x_clean, np.float32)
    ablation_mask = np.asarray(ablation_mask, np.float32)
    WqT = np.ascontiguousarray(np.asarray(Wq, np.float32).T)
    WkT = np.ascontiguousarray(np.asarray(Wk, np.float32).T)
    WvT = np.ascontiguousarray(np.asarray(Wv, np.float32).T)
    WoT = np.ascontiguousarray(np.asarray(Wo, np.float32).T)
    bo = np.asarray(bo, np.float32)

    ident = np.eye(128, dtype=ml_dtypes.bfloat16)
    pmask = np.zeros((1, 2, 128), np.float32)
    pmask[0, 0, 0:64] = 1.0
    pmask[0, 1, 64:128] = 1.0

    # masks: include iff 1 <= kt*128 + r - a <= 256 ; set 1 adds qp==0 edge
    r = np.arange(128)[:, None]
    a = np.arange(256)[None, :]
    masks_by_half = []
    for half in range(2):
        m = np.empty((2, NKT, 128, 256), np.float32)
        for kt in range(NKT):
            d = kt * 128 + r - a
            inc = (d >= 1) & (d <= 256)
            m[0, kt] = np.where(inc, 0.0, -1e30)
            inc_edge = inc & ((kt * 128 + r) >= 256) if half == 0 else inc
            m[1, kt] = np.where(inc_edge, 0.0, -1e30)
        masks_by_half.append(m.astype(ml_dtypes.bfloat16))

    in_maps = []
    for c in range(NC):
        b, half = c // 2, c % 2
        s0 = half * SL
        xTc = np.ascontiguousarray(x[b, s0 : s0 + SL].T)
        xc = np.zeros((SKL, H), np.float32)
        lo = max(0, s0 - W)
        xc[W - (s0 - lo) :] = x_clean[b, lo : s0 + SL]
        xcTc = np.ascontiguousarray(xc.T)
        ablTc = np.ascontiguousarray(ablation_mask[b, s0 : s0 + SL].T)
        in_maps.append(
            {
                "xT": xTc,
                "xcT": xcTc,
                "ablT": ablTc,
                "WqT": WqT,
                "WkT": WkT,
                "WvT": WvT,
                "WoT": WoT,
                "bo": bo,
                "masks": masks_by_half[half],
                "ident_in": ident,
                "pmask_in": pmask,
            }
        )

    if _compiled is None:
        _compiled = _build()
    res = run_bass_kernel_spmd(
        _compiled, in_maps, core_ids=list(range(NC)), trace=False
    )

    out = np.empty((B, S, H), np.float32)
    for c in range(NC):
        b, half = c // 2, c % 2
        out[b, half * SL : (half + 1) * SL] = (
            res.results[c]["outT"].T.astype(np.float32)
        )
    return out


# revision 20
# speedup vs baseline: 1.1949x; 1.0005x over previous
"""AttentionWithSelfAblation TRN2 kernel.

Reference computation (B=4, S=2048, H=1024, nh=16, hd=64, window=256):
    q = x @ Wq.T ; k = x_clean @ Wk.T ; v = x_clean @ Wv.T   (per-head split)
    scores = q @ k.T  (NO 1/sqrt(hd) scaling)
    local causal mask: key j visible to query i iff i-255 <= j <= i
    attn = softmax(scores) ; ctx = attn @ v  (merge heads)
    out = (ctx * ablation_mask) @ Wo.T + bo

Sharding: pure data/sequence parallel over 8 cores: core c = (batch c//2,
sequence half c%2 of 1024 queries). Keys/values need a 256-halo to the left;
the first half uses zero-padding + masks instead. No collectives.

Per-core device pipeline (weights stream as halves, 3-deep rotation so the
next phase's weights prefetch during the current phase's compute):
  phase Q : xT chunks streamed -> qT[o,s] f32r
  phase KV: xcT chunks streamed (twice, once per weight half) ->
            kT[o,s] f32r + v[s,o] bf16 (o augmented with ones per head)
  phase A : per (qpair of 256 queries, head): scoresT[sk,sq] =
            maskneg (PE-injected bf16) + kT_h.T @ qT_h (f32r) ; exp on ACT
            -> bf16 ; ctx-MM v.T @ exp (aug ones row = denominator) ;
            denominator copy on ACT, reciprocal + K=1 broadcast-MMs ;
            normalize * ablation (DVE) ; out-proj (PE) + bias (ACT) -> bf16.

The q/k score path stays f32r end-to-end (bf16 there fails the 2e-2 gate;
mixed bf16-stationary x f32r-moving matmul is rejected by walrus).
Host does all layout transposes/casts (free) and unshards by concatenation.
"""

import numpy as np
import ml_dtypes

from concourse import bacc
import concourse.tile as tile
import concourse.mybir as mybir
from concourse.bass_utils import run_bass_kernel_spmd

B, S, H = 4, 2048, 1024
NH, HD = 16, 64
W = 256  # window
SL = 1024  # per-core sequence chunk
SKL = SL + W  # keys incl halo
NQP = SL // 256  # qpairs of 256 queries
NKT = 4  # k-tiles of 128 per qpair
NC = 8  # cores

F32 = mybir.dt.float32
F32R = mybir.dt.float32r
BF16 = mybir.dt.bfloat16
EXP = mybir.ActivationFunctionType.Exp
IDENT = mybir.ActivationFunctionType.Identity

_compiled = None


def _build():
    nc = bacc.Bacc("TRN2", target_bir_lowering=False, debug=False)

    xT = nc.dram_tensor("xT", [H, SL], F32R, kind="ExternalInput")
    xcT = nc.dram_tensor("xcT", [H, SKL], F32R, kind="ExternalInput")
    ablT = nc.dram_tensor("ablT", [H, SL], F32, kind="ExternalInput")
    WqT = nc.dram_tensor("WqT", [H, H], F32R, kind="ExternalInput")
    WkT = nc.dram_tensor("WkT", [H, H], F32R, kind="ExternalInput")
    WvT = nc.dram_tensor("WvT", [H, H], F32R, kind="ExternalInput")
    WoT = nc.dram_tensor("WoT", [H, H], F32R, kind="ExternalInput")
    bo = nc.dram_tensor("bo", [H], F32, kind="ExternalInput")
    # masks[set, kt, sk, sq]: set 0 = standard (qp>=1), set 1 = qp==0 variant
    masks = nc.dram_tensor("masks", [2, NKT, 128, 256], BF16, kind="ExternalInput")
    ident_in = nc.dram_tensor("ident_in", [128, 128], BF16, kind="ExternalInput")
    pmask_in = nc.dram_tensor("pmask_in", [1, 2, 128], F32R, kind="ExternalInput")
    outT = nc.dram_tensor("outT", [H, SL], BF16, kind="ExternalOutput")

    xT_d = xT.rearrange("(c p) s -> p c s", p=128)
    xcT_d = xcT.rearrange("(c p) s -> p c s", p=128)
    ablT_d = ablT.rearrange("(t p) s -> p t s", p=128)
    outT_d = outT.rearrange("(t p) s -> p t s", p=128)

    with tile.TileContext(nc) as tc:
        with (
            tc.tile_pool(name="consts", bufs=1) as consts,
            tc.tile_pool(name="big", bufs=1) as big,
            tc.tile_pool(name="wpool", bufs=3) as wpool,
            tc.tile_pool(name="outp", bufs=4) as outpool,
            tc.tile_pool(name="ps512", bufs=2, space="PSUM") as ps512,
        ):

            qT_sb = big.tile([128, 8, SL], F32R)
            kT_sb = big.tile([128, 8, SKL], F32R)
            v_sb = big.tile([128, 10, 16 * 65], BF16)
            xc0_sb = big.tile([128, 8, 256], F32R, name="xc0")

            def load_weight_half(dram, hf):
                """o-columns [hf*512, (hf+1)*512) of a transposed weight."""
                w_sb = wpool.tile(
                    [128, 8, 512], F32R, name=f"w_{dram.name}_{hf}", tag="w"
                )
                for c in range(8):
                    nc.sync.dma_start(
                        w_sb[:, c, :],
                        dram.rearrange("(c p) o -> p c o", p=128)[
                            :, c, hf * 512 : (hf + 1) * 512
                        ],
                    )
                return w_sb

            # ones columns of the augmented v (slot 64 of each head's 65):
            # memset a contiguous scratch then strided-copy into place
            v_aug = v_sb[:].rearrange("p t (h e) -> p t h e", e=65)
            ones_scratch = consts.tile([128, 160], BF16)
            nc.vector.memset(ones_scratch[:], 1.0)
            nc.vector.tensor_copy(
                v_aug[:, :, :, 64],
                ones_scratch[:].rearrange("p (t h) -> p t h", t=10),
            )

            # ---- phases Q + KV share one streaming pool: the first xc
            # chunk's DMA starts as soon as an x buffer frees mid-phase-Q ----
            with tc.tile_pool(name="xs", bufs=2) as xspool:
                for hf in range(2):
                    wq_sb = load_weight_half(WqT, hf)
                    for ci in range(SL // 512):
                        x_s = xspool.tile(
                            [128, 8, 512], F32R, name=f"x_{hf}_{ci}", tag="xs"
                        )
                        for c in range(8):
                            nc.sync.dma_start(
                                x_s[:, c, :], xT_d[:, c, ci * 512 : (ci + 1) * 512]
                            )
                        if hf == 0 and ci == 0:
                            # c-major over 4 banks: compute starts after the
                            # first c-chunk lands instead of the whole wave
                            with tc.tile_pool(
                                name="psq", bufs=1, space="PSUM"
                            ) as psqp:
                                psQ = psqp.tile([128, 4, 512], F32, name="psq0")
                                for c in range(8):
                                    for oi in range(4):
                                        nc.tensor.matmul(
                                            psQ[:, oi, :],
                                            wq_sb[:, c, oi * 128 : (oi + 1) * 128],
                                            x_s[:, c, :],
                                            start=(c == 0),
                                            stop=(c == 7),
                                            skip_group_check=True,
                                        )
                                for oi in range(4):
                                    nc.vector.tensor_copy(
                                        qT_sb[:, oi, 0:512], psQ[:, oi, :]
                                    )
                        else:
                          for oi in range(4):
                            ot = hf * 4 + oi
                            ps = ps512.tile([128, 512], F32, tag="ps512")
                            for c in range(8):
                                nc.tensor.matmul(
                                    ps[:],
                                    wq_sb[:, c, oi * 128 : (oi + 1) * 128],
                                    x_s[:, c, :],
                                    start=(c == 0),
                                    stop=(c == 7),
                                )
                            nc.vector.tensor_copy(
                                qT_sb[:, ot, ci * 512 : (ci + 1) * 512], ps[:]
                            )

                # first KV chunk rides along during phase-Q compute, when
                # the DMA queues have spare bandwidth
                for c in range(8):
                    nc.sync.dma_start(xc0_sb[:, c, :], xcT_d[:, c, 1024:1280])

                # ---- phase KV: kT[o,s] f32r + v[s,o] bf16 (o aug per head) ----
                kv_chunks = [(1024, 256), (0, 512), (512, 512)]
                for hf in range(2):
                    wk_sb = load_weight_half(WkT, hf)
                    wv_sb = load_weight_half(WvT, hf)
                    for ci, (s0c, snc) in enumerate(kv_chunks):
                        if ci == 0:
                            xc_s = xc0_sb  # preloaded during phase Q
                        else:
                            xc_s = xspool.tile(
                                [128, 8, 512], F32R, name=f"xc_{hf}_{ci}", tag="xs"
                            )
                            for c in range(8):
                                nc.sync.dma_start(
                                    xc_s[:, c, :snc], xcT_d[:, c, s0c : s0c + snc]
                                )
                        for oi in range(4):
                            ot = hf * 4 + oi
                            ps = ps512.tile([128, 512], F32, tag="ps512")
                            for c in range(8):
                                nc.tensor.matmul(
                                    ps[:, :snc],
                                    wk_sb[:, c, oi * 128 : (oi + 1) * 128],
                                    xc_s[:, c, :snc],
                                    start=(c == 0),
                                    stop=(c == 7),
                                )
                            nc.scalar.copy(
                                kT_sb[:, ot, s0c : s0c + snc], ps[:, :snc]
                            )
                        for sti in range(snc // 128):
                            st = s0c // 128 + sti
                            ps = ps512.tile([128, 512], F32, tag="ps512")
                            for c in range(8):
                                nc.tensor.matmul(
                                    ps[:],
                                    xc_s[:, c, sti * 128 : (sti + 1) * 128],
                                    wv_sb[:, c, :],
                                    start=(c == 0),
                                    stop=(c == 7),
                                )
                            nc.scalar.copy(
                                v_aug[:, st, hf * 8 : (hf + 1) * 8, 0:64],
                                ps[:].rearrange("p (h e) -> p h e", e=64),
                            )

            # ---- attention constants (loaded late: not needed before phase A,
            # and early emission delays the first projection DMAs) ----
            ident = consts.tile([128, 128], BF16)
            nc.sync.dma_start(ident[:], ident_in[:])
            pmask = consts.tile([1, 2, 128], F32R)
            nc.sync.dma_start(pmask[:], pmask_in[:])
            bo_sb = consts.tile([128, 8], F32)
            nc.sync.dma_start(bo_sb[:], bo.rearrange("(t p) -> p t", p=128))
            mask_sb = consts.tile([128, 2, NKT, 256], BF16)
            for ms in range(2):
                nc.sync.dma_start(
                    mask_sb[:, ms, :, :],
                    masks.rearrange("s t k q -> k s t q")[:, ms, :, :],
                )

            # ---- phase A: attention + out-projection per qpair ----
            wo_hs = [load_weight_half(WoT, hf) for hf in range(2)]
            with (
                tc.tile_pool(name="exp", bufs=4) as exppool,
                tc.tile_pool(name="recip", bufs=3) as recippool,
                tc.tile_pool(name="abl", bufs=4) as ablpool,
                tc.tile_pool(name="ctxs", bufs=1) as ctxpool,
                tc.tile_pool(name="ps_sc", bufs=2, space="PSUM") as ps_sc,
                tc.tile_pool(name="ps_ctx", bufs=2, space="PSUM") as ps_ctx,
            ):
                for qg in range(NQP // 2):
                  ctx_sb = ctxpool.tile(
                      [128, 8, 512], F32R, name=f"ctx_{qg}", tag="ctx"
                  )
                  for qph in range(2):
                    qp = qg * 2 + qph
                    qsl = slice(qph * 256, qph * 256 + 256)
                    ms = 1 if qp == 0 else 0
                    for t in range(NH // 2):  # head pair
                        pss = [
                            ps_sc.tile(
                                [128, NKT, 256], F32,
                                name=f"sc_{qp}_{2 * t + par}", tag="sc",
                            )
                            for par in range(2)
                        ]
                        exps = []
                        # per par: inject mask, qk matmuls, then exp --
                        # exp of par 0 overlaps the par-1 matmuls, and the
                        # even/odd 64-row groups overlap on the PE
                        for par in range(2):
                            ps = pss[par]
                            for kg in range(2):  # one mask inject per 2 k-tiles
                                nc.tensor.matmul(
                                    ps[:, kg * 2 : kg * 2 + 2, :],
                                    ident[:],
                                    mask_sb[:, ms, kg * 2 : kg * 2 + 2, :],
                                    start=True,
                                    stop=False,
                                    skip_group_check=True,
                                )
                            hsl = slice(par * 64, par * 64 + 64)
                            for kt in range(NKT):
                                lj0 = qp * 256 + kt * 128
                                nc.tensor.matmul(
                                    ps[:, kt, :],
                                    kT_sb[hsl, t, lj0 : lj0 + 128],
                                    qT_sb[hsl, t, qp * 256 : qp * 256 + 256],
                                    start=False,
                                    stop=True,
                                    skip_group_check=True,
                                )
                            exp_sb = exppool.tile(
                                [128, NKT, 256], BF16,
                                name=f"exp_{qp}_{2 * t + par}", tag="exp",
                            )
                            exps.append(exp_sb)
                            nc.scalar.activation(exp_sb[:], ps[:], EXP)
                        psc = ps_ctx.tile(
                            [65, 2, 256], F32, name=f"ctxp_{qp}_{t}", tag="ctxp"
                        )
                        for par in range(2):
                            h = 2 * t + par
                            for kt in range(NKT):
                                nc.tensor.matmul(
                                    psc[:, par, :],
                                    v_sb[:, qp * 2 + kt, h * 65 : h * 65 + 65],
                                    exps[par][:, kt, :],
                                    start=(kt == 0),
                                    stop=(kt == NKT - 1),
                                )
                        recf = recippool.tile(
                            [1, 2, 512], F32, name=f"recf_{qp}_{t}", tag="recf"
                        )
                        rec = recippool.tile(
                            [1, 2, 256], F32R, name=f"rec_{qp}_{t}", tag="rec"
                        )
                        # denominator copy off PSUM on ACT (frees the DVE)
                        nc.scalar.copy(
                            recf[:, 0, :].rearrange("p (a q) -> p a q", a=2),
                            psc[64:65, :, :],
                        )
                        nc.vector.reciprocal_approx_fast(recf[:, 1, :], recf[:, 0, :])
                        nc.vector.tensor_copy(
                            rec[:], recf[:, 1, :].rearrange("p (a q) -> p a q", a=2)
                        )
                        # drain pair: even head -> parts 0:64, odd -> 64:128
                        nc.vector.tensor_copy(ctx_sb[0:64, t, qsl], psc[0:64, 0, :])
                        nc.vector.tensor_copy(ctx_sb[64:128, t, qsl], psc[0:64, 1, :])
                        # normalize + ablate inline (spreads the PE bcast MMs
                        # across the t loop instead of bunching at qp end)
                        psb = ps512.tile(
                            [128, 512], F32, name=f"bc_{qp}_{t}", tag="ps512"
                        )
                        for par in range(2):
                            nc.tensor.matmul(
                                psb[:, :256],
                                pmask[:, par, :],
                                rec[:, par, :],
                                start=(par == 0),
                                stop=(par == 1),
                            )
                        abl_sb = ablpool.tile(
                            [128, 256], F32, name=f"abl_{qp}_{t}", tag="abl"
                        )
                        nc.sync.dma_start(
                            abl_sb[:], ablT_d[:, t, qp * 256 : qp * 256 + 256]
                        )
                        nc.vector.tensor_mul(
                            ctx_sb[:, t, qsl], ctx_sb[:, t, qsl], psb[:, :256]
                        )
                        nc.vector.tensor_mul(
                            ctx_sb[:, t, qsl], ctx_sb[:, t, qsl], abl_sb[:]
                        )

                  # out projection for this pair of qpairs (N=512)
                  for ot in range(8):
                      wo_sb = wo_hs[ot // 4]
                      oi = ot % 4
                      ps = ps512.tile(
                          [128, 512], F32, name=f"op_{qg}_{ot}", tag="ps512"
                      )
                      for c in range(8):
                          nc.tensor.matmul(
                              ps[:],
                              wo_sb[:, c, oi * 128 : (oi + 1) * 128],
                              ctx_sb[:, c, :],
                              start=(c == 0),
                              stop=(c == 7),
                          )
                      o_sb = outpool.tile(
                          [128, 512], BF16, name=f"out_{qg}_{ot}", tag="outp"
                      )
                      nc.scalar.activation(
                          o_sb[:], ps[:], IDENT, bias=bo_sb[:, ot : ot + 1]
                      )
                      nc.sync.dma_start(
                          outT_d[:, ot, qg * 512 : qg * 512 + 512], o_sb[:]
                      )
    nc.compile()
    return nc


def kernel(x, x_clean, ablation_mask, Wq, Wk, Wv, Wo, bo):
    global _compiled
    x = np.asarray(x, np.float32)
    x_clean = np.asarray(x_clean, np.float32)
    ablation_mask = np.asarray(ablation_mask, np.float32)
    WqT = np.ascontiguousarray(np.asarray(Wq, np.float32).T)
    WkT = np.ascontiguousarray(np.asarray(Wk, np.float32).T)
    WvT = np.ascontiguousarray(np.asarray(Wv, np.float32).T)
    WoT = np.ascontiguousarray(np.asarray(Wo, np.float32).T)
    bo = np.asarray(bo, np.float32)

    ident = np.eye(128, dtype=ml_dtypes.bfloat16)
    pmask = np.zeros((1, 2, 128), np.float32)
    pmask[0, 0, 0:64] = 1.0
    pmask[0, 1, 64:128] = 1.0

    # masks: include iff 1 <= kt*128 + r - a <= 256 ; set 1 adds qp==0 edge
    r = np.arange(128)[:, None]
    a = np.arange(256)[None, :]
    masks_by_half = []
    for half in range(2):
        m = np.empty((2, NKT, 128, 256), np.float32)
        for kt in range(NKT):
            d = kt * 128 + r - a
            inc = (d >= 1) & (d <= 256)
            m[0, kt] = np.where(inc, 0.0, -1e30)
            inc_edge = inc & ((kt * 128 + r) >= 256) if half == 0 else inc
            m[1, kt] = np.where(inc_edge, 0.0, -1e30)
        masks_by_half.append(m.astype(ml_dtypes.bfloat16))

    in_maps = []
    for c in range(NC):
        b, half = c // 2, c % 2
        s0 = half * SL
        xTc = np.ascontiguousarray(x[b, s0 : s0 + SL].T)
        xc = np.zeros((SKL, H), np.float32)
        lo = max(0, s0 - W)
        xc[W - (s0 - lo) :] = x_clean[b, lo : s0 + SL]
        xcTc = np.ascontiguousarray(xc.T)
        ablTc = np.ascontiguousarray(ablation_mask[b, s0 : s0 + SL].T)
        in_maps.append(
            {
                "xT": xTc,
                "xcT": xcTc,
                "ablT": ablTc,
                "WqT": WqT,
                "WkT": WkT,
                "WvT": WvT,
                "WoT": WoT,
                "bo": bo,
                "masks": masks_by_half[half],
                "ident_in": ident,
                "pmask_in": pmask,
            }
        )

    if _compiled is None:
        _compiled = _build()
    res = run_bass_kernel_spmd(
        _compiled, in_maps, core_ids=list(range(NC)), trace=False
    )

    out = np.empty((B, S, H), np.float32)
    for c in range(NC):
        b, half = c // 2, c % 2
        out[b, half * SL : (half + 1) * SL] = (
            res.results[c]["outT"].T.astype(np.float32)
        )
    return out
